# revision 17
# baseline (speedup 1.0000x reference)
"""Chamfer distance (nn_ChamferLossLayer) on 8 Trainium2 NeuronCores.

Two-phase grid-retrieval design (arch_category: retrieval_knn):

  Host builds a spatial index (pure marshaling -- no point-to-point
  distances): a 16^3 per-axis-quantile grid over each cloud. Coordinates
  are N(0,1) i.i.d., so quantile cells have near-uniform occupancy
  (~2.9 points/cell). Queries are grouped by 4x4x2-cell blocks (<=128
  queries each, the PE stationary width); each block's candidate set is
  its block+1-ring cells (<=512 points, one PE moving chunk / one PSUM
  bank).

  Device pass (SPMD on 8 cores): one K=11 augmented bf16 matmul
  (hi/mid split operands, f32 PSUM accumulate) computes
  D'[q, c] = |c|^2 - 2<q, c> for a 128-query x <=512-candidate tile,
  then one DVE tensor_reduce(min) gives each query's candidate min.
  Up to 4 passes share one [128, 4, 512] PSUM tile so a single reduce
  covers 4 passes (amortizes PSUM-access + dispatch overhead). Passes
  are sorted by candidate count and dealt round-robin to cores, so each
  slot band gets a tight compile-time width Kg (max count in band,
  rounded up to 16) -- the reduce/matmul only touch Kg of the 512 slot
  columns. min_c D = min_c D' + |q|^2: the query norm is added on host.

  Phase 1 covers every query with its block+ring candidates. The host
  then certifies each query geometrically: if d1 <= dist^2(q, outside
  of the block+ring box), no other point can be closer -- exact. The
  ~3% failing (tail) queries go to phase 2: candidate cells are all
  cells with mindist^2(q, cell) <= d1, grouped greedily. This makes the
  result exact (up to split-precision ~1e-4 relative, vs the 2e-2 gate)
  with ~1-2 extra reduce groups per core.

  Per-core cost is DVE-bound: sum over groups of ~4*Kg*1.04ns + fixed.
"""

import numpy as np
import ml_dtypes

import concourse.bacc as bacc
import concourse.mybir as mybir
from concourse.bass_utils import run_bass_kernel_spmd
from concourse.tile import TileContext

F32 = mybir.dt.float32
BF16 = mybir.dt.bfloat16
BF = ml_dtypes.bfloat16

N_CORES = 8
N, P = 2, 12000            # batches, points per cloud
G = 16                     # grid cells per axis
BS = (4, 4, 2)             # block shape in cells
NB = (G // BS[0], G // BS[1], G // BS[2])
K = 512                    # candidate slot columns per pass (max Kg)
Q = 128                    # query slots per pass (PE stationary width)
KAUG = 11                  # augmented contraction: 9 coord splits + sq2 hi/mid
W = Q + K                  # combined stationary+moving slot width
BIG = 32768.0              # pad distance (exact in bf16), >> any real D'

# N(0,1) quantiles i/16, i=1..15
EDGES = np.array([
    -1.5341205443525463, -1.1503493803760083, -0.8871465590189085,
    -0.6744897501960817, -0.4887764111146693, -0.3186393639643751,
    -0.15731068461017067, 0.0, 0.15731068461017067, 0.3186393639643751,
    0.4887764111146693, 0.6744897501960817, 0.8871465590189085,
    1.1503493803760083, 1.5341205443525463])
LO = np.concatenate([[-np.inf], EDGES])
HI = np.concatenate([EDGES, [np.inf]])

_PROGRAMS: dict[tuple, object] = {}
LAST_RUN_PROGRAMS: list = []


def _build_program(kgs):
    """One SPMD program for a group schedule. `kgs` is a tuple of
    (group_size <= 4, Kg <= 512) per reduce group; slots are consecutive."""
    if kgs in _PROGRAMS:
        return _PROGRAMS[kgs]
    nslots = sum(gs for gs, _ in kgs)
    nc = bacc.Bacc()
    # stationary ([:, :, :Q]) and moving ([:, :, Q:]) share one input tensor
    sm = nc.dram_tensor("sm", [KAUG, nslots, W], BF16, kind="ExternalInput")
    res = nc.dram_tensor("res", [Q, nslots], F32, kind="ExternalOutput")
    # chunk 0 is small so the first matmul group starts ASAP
    bounds = [0, min(4, nslots)]
    while bounds[-1] < nslots:
        bounds.append(min(nslots, bounds[-1] + max(8, (nslots - 4) // 3)))
    half = len(kgs) // 2
    with TileContext(nc) as tc:
        with tc.tile_pool(name="sbuf", bufs=1) as pool, \
             tc.tile_pool(name="psum", bufs=1, space="PSUM") as pp:
            sm_sb = pool.tile([KAUG, nslots, W], BF16, tag="sm", name="sm_sb")
            r_sb = pool.tile([Q, nslots], F32, tag="r", name="r_sb")
            ps = [pp.tile([Q, 4, K], F32, name=f"ps{k}", tag=f"ps{k}")
                  for k in range(2)]
            for a, b in zip(bounds[:-1], bounds[1:]):
                nc.sync.dma_start(out=sm_sb[:, a:b, :], in_=sm[:, a:b, :])
            # PE p-state warmup: ~48 back-to-back 64-wide dummy matmuls into
            # disjoint PSUM slices while the first input chunk is in flight,
            # so real matmuls run at full clock from the start
            wu = pool.tile([Q, 192], BF16, tag="wu", name="wu")
            nc.gpsimd.memset(wu[:, :], 0.0)
            for d in range(48):
                pk = ps[d % 2]
                j = (d // 2) % 4
                c0 = 64 * ((d // 8) % 8)
                nc.tensor.matmul(
                    pk[:, j, c0:c0 + 64],
                    wu[:, 0:128],
                    wu[:, 128:192],
                    start=True, stop=True)
            s0 = 0
            for g, (gs, kg) in enumerate(kgs):
                pk = ps[g % 2]
                for j in range(gs):
                    p = s0 + j
                    nc.tensor.matmul(
                        pk[:, j, 0:kg],
                        sm_sb[:, p, 0:Q],
                        sm_sb[:, p, Q:Q + kg],
                        start=True, stop=True)
                nc.vector.tensor_reduce(
                    out=r_sb[:, s0:s0 + gs], in_=pk[:, 0:gs, 0:kg],
                    op=mybir.AluOpType.min, axis=mybir.AxisListType.X)
                s0 += gs
                if g == half and len(kgs) > 2:
                    nc.sync.dma_start(out=res[:, 0:s0], in_=r_sb[:, 0:s0])
            hs = sum(gs for gs, _ in kgs[:half + 1]) if len(kgs) > 2 else 0
            nc.sync.dma_start(out=res[:, hs:], in_=r_sb[:, hs:])
    nc.finalize()
    _PROGRAMS[kgs] = nc
    return nc


def _cell_ids(X):
    return np.stack([np.searchsorted(EDGES, X[:, d]) for d in range(3)], 1)


def _split2(x):
    hi = x.astype(BF)
    mid = (x - hi.astype(np.float32)).astype(BF)
    return hi, mid


class _Aug:
    """Precomputed augmented bf16 operand rows for one problem (A->B)."""

    def __init__(self, A, B):
        qhi, qmid = _split2(-2.0 * A)
        chi, cmid = _split2(B)
        sq2 = np.einsum("pd,pd->p", B.astype(np.float64),
                        B.astype(np.float64)).astype(np.float32)
        s2hi, s2mid = _split2(sq2)
        # stationary rows [11, P]: qhi x3, qmid x3, qhi x3, 1, 1
        self.S = np.ones((KAUG, len(A)), BF)
        self.S[0:3] = qhi.T
        self.S[3:6] = qmid.T
        self.S[6:9] = qhi.T
        # moving rows [11, P]: chi x3, chi x3, cmid x3, s2hi, s2mid
        self.M = np.zeros((KAUG, len(B)), BF)
        self.M[0:3] = chi.T
        self.M[3:6] = chi.T
        self.M[6:9] = cmid.T
        self.M[9] = s2hi
        self.M[10] = s2mid
        self.sq1 = np.einsum("pd,pd->p", A.astype(np.float64),
                             A.astype(np.float64)).astype(np.float32)


def _pack_phase1(A, B):
    """Block+ring passes [(qidx<=128, cidx<=512)] and per-query
    certificate radius g2 = dist^2 to outside the block+ring box."""
    ca, cb = _cell_ids(A), _cell_ids(B)
    bq = (ca[:, 0] // BS[0]) * NB[1] * NB[2] + (ca[:, 1] // BS[1]) * NB[2] \
        + (ca[:, 2] // BS[2])
    order = np.argsort(bq, kind="stable")
    bnd = np.searchsorted(bq[order], np.arange(NB[0] * NB[1] * NB[2] + 1))
    passes = []
    g2 = np.empty(len(A))
    for b in range(NB[0] * NB[1] * NB[2]):
        qi = order[bnd[b]:bnd[b + 1]]
        if len(qi) == 0:
            continue
        bx, by, bz = b // (NB[1] * NB[2]), (b // NB[2]) % NB[1], b % NB[2]
        clo = np.array([bx * BS[0] - 1, by * BS[1] - 1, bz * BS[2] - 1])
        chi = np.array([(bx + 1) * BS[0], (by + 1) * BS[1], (bz + 1) * BS[2]])
        sel = np.ones(len(B), bool)
        for d in range(3):
            sel &= (cb[:, d] >= clo[d]) & (cb[:, d] <= chi[d])
        ci = np.where(sel)[0]
        blo = np.array([LO[clo[d]] if clo[d] >= 0 else -np.inf
                        for d in range(3)])
        bhi = np.array([HI[chi[d]] if chi[d] <= G - 1 else np.inf
                        for d in range(3)])
        dface = np.minimum(A[qi] - blo, bhi - A[qi])
        gg = np.min(dface, 1)
        g2[qi] = np.where(np.isinf(gg), np.inf, gg * gg)
        for q0 in range(0, len(qi), Q):
            qs = qi[q0:q0 + Q]
            if len(ci) == 0:
                continue
            for c0 in range(0, len(ci), K):
                passes.append((qs, ci[c0:c0 + K]))
    return passes, g2


def _pack_phase2(A, B, fail_idx, rad2):
    """Passes covering, for each failing query, every B point in a cell
    with mindist^2(q, cell) <= rad2. Greedy grouping in cell order."""
    if len(fail_idx) == 0:
        return []
    cb = _cell_ids(B)
    occ: dict[tuple, list] = {}
    for i, c in enumerate(cb):
        occ.setdefault(tuple(c), []).append(i)
    keys = np.array(list(occ.keys()))
    counts = np.array([len(occ[tuple(k)]) for k in keys])
    klo = np.stack([LO[keys[:, d]] for d in range(3)], 1)
    khi = np.stack([HI[keys[:, d]] for d in range(3)], 1)
    need = []
    for t, qi in enumerate(fail_idx):
        q = A[qi]
        d = np.maximum(np.maximum(klo - q, 0), np.maximum(q - khi, 0))
        md2 = (d * d).sum(1)
        need.append(np.where(md2 <= rad2[t])[0])
    ca = _cell_ids(A[fail_idx])
    order = np.lexsort((ca[:, 2], ca[:, 1], ca[:, 0]))
    passes = []
    cur_q: list = []
    cur_cells: set = set()

    def flush():
        if not cur_q:
            return
        pts = np.sort(np.concatenate(
            [occ[tuple(keys[c])] for c in cur_cells]).astype(np.int64)) \
            if cur_cells else np.zeros(0, np.int64)
        qs = fail_idx[np.array(cur_q)]
        for c0 in range(0, len(pts), K):
            passes.append((qs, pts[c0:c0 + K]))

    qcap = Q
    for t in order:
        u = cur_cells | set(need[t])
        npts = counts[list(u)].sum() if u else 0
        if len(cur_q) + 1 > qcap or (npts > K and len(cur_q) > 0):
            flush()
            cur_q, cur_cells = [], set()
            u = set(need[t])
        cur_q.append(t)
        cur_cells = u
    flush()
    return passes


def _dispatch(tagged_passes, augs):
    """Run one SPMD dispatch covering `tagged_passes` [(pid, qidx, cidx)].
    Returns per-problem arrays of min D' (f32, +inf where untouched)."""
    # sort by candidate count (asc) and deal round-robin: each 4-slot
    # band then shares a tight compile-time width Kg, and the pipeline
    # fills with small groups while the PE p-state is still ramping
    order = np.argsort([len(cs) for _, _, cs in tagged_passes],
                       kind="stable")
    nslots = -(-len(order) // N_CORES)
    # Kg per slot group: max count in the band, rounded up to 16. Leading
    # groups are small (1,1,2) so the first reduces fire ASAP during the
    # DMA/p-state fill; the rest are 4 wide (one PSUM tile).
    def band_max(s0, gs):
        mx = 1
        for sl in range(s0, s0 + gs):
            for r in order[sl * N_CORES:(sl + 1) * N_CORES]:
                mx = max(mx, len(tagged_passes[r][2]))
        return -(-mx // 16) * 16
    kgs = []
    s0 = 0
    for gs in (1, 1, 2):
        if s0 >= nslots:
            break
        gs = min(gs, nslots - s0)
        kgs.append((gs, band_max(s0, gs)))
        s0 += gs
    while s0 < nslots:
        gs = min(4, nslots - s0)
        kgs.append((gs, band_max(s0, gs)))
        s0 += gs
    nc = _build_program(tuple(kgs))
    SM = np.zeros((N_CORES, KAUG, nslots, W), BF)
    SM[:, 9:11, :, 0:Q] = BF(1.0)   # stationary ones rows
    SM[:, 9, :, Q:] = BF(np.float32(BIG))  # pad candidates: D' = BIG
    meta = []
    for i, r in enumerate(order):
        pid, qs, cs = tagged_passes[r]
        c, sl = i % N_CORES, i // N_CORES
        a = augs[pid]
        SM[c, :, sl, 0:len(qs)] = a.S[:, qs]
        SM[c, :, sl, Q:Q + len(cs)] = a.M[:, cs]
        meta.append((c, sl, pid, qs))
    in_maps = [{"sm": np.ascontiguousarray(SM[c])} for c in range(N_CORES)]
    br = run_bass_kernel_spmd(nc, in_maps, list(range(N_CORES)))
    LAST_RUN_PROGRAMS.append(nc)
    outs = [np.full(len(a.sq1), np.inf, np.float32) for a in augs]
    for c, sl, pid, qs in meta:
        np.minimum.at(outs[pid], qs, br.results[c]["res"][:len(qs), sl])
    return outs


def kernel(cloud1, cloud2):
    c1 = np.asarray(cloud1, np.float32)
    c2 = np.asarray(cloud2, np.float32)
    LAST_RUN_PROGRAMS.clear()
    # problems: (batch0 1->2, 2->1, batch1 1->2, 2->1)
    probs = []
    for n in range(N):
        probs.append((c1[n], c2[n]))
        probs.append((c2[n], c1[n]))
    augs = [_Aug(A, B) for A, B in probs]

    p1_tagged = []
    g2s = []
    for pid, (A, B) in enumerate(probs):
        passes, g2 = _pack_phase1(A, B)
        g2s.append(g2)
        p1_tagged += [(pid, qs, cs) for qs, cs in passes]
    d1p = _dispatch(p1_tagged, augs)

    p2_tagged = []
    for pid, (A, B) in enumerate(probs):
        d1 = d1p[pid] + augs[pid].sq1
        margin = np.maximum(1e-3 * np.abs(d1), 1e-6)
        fail_idx = np.where(~(d1 + margin <= g2s[pid]))[0]
        rad2 = (d1[fail_idx] + margin[fail_idx]).astype(np.float64)
        p2_tagged += [(pid, qs, cs)
                      for qs, cs in _pack_phase2(A, B, fail_idx, rad2)]
    if p2_tagged:
        d2p = _dispatch(p2_tagged, augs)
    else:
        d2p = [np.full(len(a.sq1), np.inf, np.float32) for a in augs]

    terms = []
    for pid in range(len(probs)):
        dmin = np.minimum(d1p[pid], d2p[pid]) + augs[pid].sq1
        terms.append(dmin.astype(np.float64).mean())
    out = np.array([terms[2 * n] + terms[2 * n + 1] for n in range(N)],
                   np.float32)
    return out


# revision 21
# speedup vs baseline: 1.0442x; 1.0442x over previous
"""Chamfer distance (nn_ChamferLossLayer) on 8 Trainium2 NeuronCores.

Two-phase grid-retrieval design (arch_category: retrieval_knn):

  Host builds a spatial index (pure marshaling -- no point-to-point
  distances): a 16^3 per-axis-quantile grid over each cloud. Coordinates
  are N(0,1) i.i.d., so quantile cells have near-uniform occupancy
  (~2.9 points/cell). Queries are grouped by 4x4x2-cell blocks (<=128
  queries each, the PE stationary width); each block's candidate set is
  its block+1-ring cells (<=512 points, one PE moving chunk / one PSUM
  bank).

  Device pass (SPMD on 8 cores): one K=11 augmented bf16 matmul
  (hi/mid split operands, f32 PSUM accumulate) computes
  D'[q, c] = |c|^2 - 2<q, c> for a 128-query x <=512-candidate tile,
  then one DVE tensor_reduce(min) gives each query's candidate min.
  Up to 4 passes share one [128, 4, 512] PSUM tile so a single reduce
  covers 4 passes (amortizes PSUM-access + dispatch overhead). Passes
  are sorted by candidate count and dealt round-robin to cores, so each
  slot band gets a tight compile-time width Kg (max count in band,
  rounded up to 16) -- the reduce/matmul only touch Kg of the 512 slot
  columns. min_c D = min_c D' + |q|^2: the query norm is added on host.

  Phase 1 covers every query with its block+ring candidates. The host
  then certifies each query geometrically: if d1 <= dist^2(q, outside
  of the block+ring box), no other point can be closer -- exact. The
  ~3% failing (tail) queries go to phase 2: candidate cells are all
  cells with mindist^2(q, cell) <= d1, grouped greedily. This makes the
  result exact (up to split-precision ~1e-4 relative, vs the 2e-2 gate)
  with ~1-2 extra reduce groups per core.

  Per-core cost is DVE-bound: sum over groups of ~4*Kg*1.04ns + fixed.
"""

import numpy as np
import ml_dtypes

import concourse.bacc as bacc
import concourse.mybir as mybir
from concourse.bass_utils import run_bass_kernel_spmd
from concourse.tile import TileContext

F32 = mybir.dt.float32
BF16 = mybir.dt.bfloat16
BF = ml_dtypes.bfloat16

N_CORES = 8
N, P = 2, 12000            # batches, points per cloud
G = 16                     # grid cells per axis
BS = (4, 4, 2)             # block shape in cells
NB = (G // BS[0], G // BS[1], G // BS[2])
K = 512                    # candidate slot columns per pass (max Kg)
Q = 128                    # query slots per pass (PE stationary width)
KAUG = 11                  # augmented contraction: 9 coord splits + sq2 hi/mid
W = Q + K                  # combined stationary+moving slot width
BIG = 32768.0              # pad distance (exact in bf16), >> any real D'

# N(0,1) quantiles i/16, i=1..15
EDGES = np.array([
    -1.5341205443525463, -1.1503493803760083, -0.8871465590189085,
    -0.6744897501960817, -0.4887764111146693, -0.3186393639643751,
    -0.15731068461017067, 0.0, 0.15731068461017067, 0.3186393639643751,
    0.4887764111146693, 0.6744897501960817, 0.8871465590189085,
    1.1503493803760083, 1.5341205443525463])
LO = np.concatenate([[-np.inf], EDGES])
HI = np.concatenate([EDGES, [np.inf]])

_PROGRAMS: dict[tuple, object] = {}
LAST_RUN_PROGRAMS: list = []


def _build_program(kgs):
    """One SPMD program for a group schedule. `kgs` is a tuple of
    (group_size <= 4, Kg <= 512) per reduce group; slots are consecutive."""
    if kgs in _PROGRAMS:
        return _PROGRAMS[kgs]
    nslots = sum(gs for gs, _ in kgs)
    nc = bacc.Bacc()
    # stationary ([:, :, :Q]) and moving ([:, :, Q:]) share one input tensor
    sm = nc.dram_tensor("sm", [KAUG, nslots, W], BF16, kind="ExternalInput")
    res = nc.dram_tensor("res", [Q, nslots], F32, kind="ExternalOutput")
    # chunk 0 is small so the first matmul group starts ASAP
    bounds = [0, min(4, nslots)]
    step = max(8, -(-(nslots - bounds[-1]) // 3))
    while bounds[-1] < nslots:
        bounds.append(min(nslots, bounds[-1] + step))
    half = len(kgs) // 2
    with TileContext(nc) as tc:
        with tc.tile_pool(name="sbuf", bufs=1) as pool, \
             tc.tile_pool(name="psum", bufs=1, space="PSUM") as pp:
            sm_sb = pool.tile([KAUG, nslots, W], BF16, tag="sm", name="sm_sb")
            r_sb = pool.tile([Q, nslots], F32, tag="r", name="r_sb")
            ps = [pp.tile([Q, 4, K], F32, name=f"ps{k}", tag=f"ps{k}")
                  for k in range(2)]
            for a, b in zip(bounds[:-1], bounds[1:]):
                nc.sync.dma_start(out=sm_sb[:, a:b, :], in_=sm[:, a:b, :])
            # PE p-state warmup: back-to-back 64-wide dummy matmuls into
            # disjoint PSUM slices while the first input chunk is in flight,
            # so real matmuls run near full clock from the start (sized to
            # end ~when the first chunk's DMA semaphore lands)
            wu = pool.tile([Q, 192], BF16, tag="wu", name="wu")
            nc.gpsimd.memset(wu[:, :], 0.0)
            for d in range(40):
                pk = ps[d % 2]
                j = (d // 2) % 4
                c0 = 64 * ((d // 8) % 8)
                nc.tensor.matmul(
                    pk[:, j, c0:c0 + 64],
                    wu[:, 0:128],
                    wu[:, 128:192],
                    start=True, stop=True)
            s0 = 0
            for g, (gs, kg) in enumerate(kgs):
                pk = ps[g % 2]
                for j in range(gs):
                    p = s0 + j
                    nc.tensor.matmul(
                        pk[:, j, 0:kg],
                        sm_sb[:, p, 0:Q],
                        sm_sb[:, p, Q:Q + kg],
                        start=True, stop=True)
                nc.vector.tensor_reduce(
                    out=r_sb[:, s0:s0 + gs], in_=pk[:, 0:gs, 0:kg],
                    op=mybir.AluOpType.min, axis=mybir.AxisListType.X)
                s0 += gs
                if g == half and len(kgs) > 2:
                    nc.sync.dma_start(out=res[:, 0:s0], in_=r_sb[:, 0:s0])
            hs = sum(gs for gs, _ in kgs[:half + 1]) if len(kgs) > 2 else 0
            nc.sync.dma_start(out=res[:, hs:], in_=r_sb[:, hs:])
    nc.finalize()
    _PROGRAMS[kgs] = nc
    return nc


def _cell_ids(X):
    return np.stack([np.searchsorted(EDGES, X[:, d]) for d in range(3)], 1)


def _split2(x):
    hi = x.astype(BF)
    mid = (x - hi.astype(np.float32)).astype(BF)
    return hi, mid


class _Aug:
    """Precomputed augmented bf16 operand rows for one problem (A->B)."""

    def __init__(self, A, B):
        qhi, qmid = _split2(-2.0 * A)
        chi, cmid = _split2(B)
        sq2 = np.einsum("pd,pd->p", B.astype(np.float64),
                        B.astype(np.float64)).astype(np.float32)
        s2hi, s2mid = _split2(sq2)
        # stationary rows [11, P]: qhi x3, qmid x3, qhi x3, 1, 1
        self.S = np.ones((KAUG, len(A)), BF)
        self.S[0:3] = qhi.T
        self.S[3:6] = qmid.T
        self.S[6:9] = qhi.T
        # moving rows [11, P]: chi x3, chi x3, cmid x3, s2hi, s2mid
        self.M = np.zeros((KAUG, len(B)), BF)
        self.M[0:3] = chi.T
        self.M[3:6] = chi.T
        self.M[6:9] = cmid.T
        self.M[9] = s2hi
        self.M[10] = s2mid
        self.sq1 = np.einsum("pd,pd->p", A.astype(np.float64),
                             A.astype(np.float64)).astype(np.float32)


def _pack_phase1(A, B):
    """Block+ring passes [(qidx<=128, cidx<=512)] and per-query
    certificate radius g2 = dist^2 to outside the block+ring box."""
    ca, cb = _cell_ids(A), _cell_ids(B)
    bq = (ca[:, 0] // BS[0]) * NB[1] * NB[2] + (ca[:, 1] // BS[1]) * NB[2] \
        + (ca[:, 2] // BS[2])
    order = np.argsort(bq, kind="stable")
    bnd = np.searchsorted(bq[order], np.arange(NB[0] * NB[1] * NB[2] + 1))
    passes = []
    g2 = np.empty(len(A))
    for b in range(NB[0] * NB[1] * NB[2]):
        qi = order[bnd[b]:bnd[b + 1]]
        if len(qi) == 0:
            continue
        bx, by, bz = b // (NB[1] * NB[2]), (b // NB[2]) % NB[1], b % NB[2]
        clo = np.array([bx * BS[0] - 1, by * BS[1] - 1, bz * BS[2] - 1])
        chi = np.array([(bx + 1) * BS[0], (by + 1) * BS[1], (bz + 1) * BS[2]])
        sel = np.ones(len(B), bool)
        for d in range(3):
            sel &= (cb[:, d] >= clo[d]) & (cb[:, d] <= chi[d])
        ci = np.where(sel)[0]
        blo = np.array([LO[clo[d]] if clo[d] >= 0 else -np.inf
                        for d in range(3)])
        bhi = np.array([HI[chi[d]] if chi[d] <= G - 1 else np.inf
                        for d in range(3)])
        dface = np.minimum(A[qi] - blo, bhi - A[qi])
        gg = np.min(dface, 1)
        g2[qi] = np.where(np.isinf(gg), np.inf, gg * gg)
        for q0 in range(0, len(qi), Q):
            qs = qi[q0:q0 + Q]
            if len(ci) == 0:
                continue
            for c0 in range(0, len(ci), K):
                passes.append((qs, ci[c0:c0 + K]))
    return passes, g2


def _pack_phase2(A, B, fail_idx, rad2):
    """Passes covering, for each failing query, every B point in a cell
    with mindist^2(q, cell) <= rad2. Greedy grouping in cell order."""
    if len(fail_idx) == 0:
        return []
    cb = _cell_ids(B)
    occ: dict[tuple, list] = {}
    for i, c in enumerate(cb):
        occ.setdefault(tuple(c), []).append(i)
    keys = np.array(list(occ.keys()))
    counts = np.array([len(occ[tuple(k)]) for k in keys])
    klo = np.stack([LO[keys[:, d]] for d in range(3)], 1)
    khi = np.stack([HI[keys[:, d]] for d in range(3)], 1)
    need = []
    for t, qi in enumerate(fail_idx):
        q = A[qi]
        d = np.maximum(np.maximum(klo - q, 0), np.maximum(q - khi, 0))
        md2 = (d * d).sum(1)
        need.append(np.where(md2 <= rad2[t])[0])
    ca = _cell_ids(A[fail_idx])
    # Morton (bit-interleaved) cell order: queries that are close in 3D
    # pack together, so their needed-cell sets overlap in each group
    morton = np.zeros(len(ca), np.int64)
    for i in range(4):
        for d in range(3):
            morton |= ((ca[:, d].astype(np.int64) >> i) & 1) << (3 * i + 2 - d)
    order = np.argsort(morton, kind="stable")
    passes = []
    cur_q: list = []
    cur_cells: set = set()

    def flush():
        if not cur_q:
            return
        pts = np.sort(np.concatenate(
            [occ[tuple(keys[c])] for c in cur_cells]).astype(np.int64)) \
            if cur_cells else np.zeros(0, np.int64)
        qs = fail_idx[np.array(cur_q)]
        for c0 in range(0, len(pts), K):
            passes.append((qs, pts[c0:c0 + K]))

    qcap = Q
    for t in order:
        u = cur_cells | set(need[t])
        npts = counts[list(u)].sum() if u else 0
        if len(cur_q) + 1 > qcap or (npts > K and len(cur_q) > 0):
            flush()
            cur_q, cur_cells = [], set()
            u = set(need[t])
        cur_q.append(t)
        cur_cells = u
    flush()
    return passes


def _dispatch(tagged_passes, augs):
    """Run one SPMD dispatch covering `tagged_passes` [(pid, qidx, cidx)].
    Returns per-problem arrays of min D' (f32, +inf where untouched)."""
    # sort by candidate count (asc) and deal round-robin: each 4-slot
    # band then shares a tight compile-time width Kg, and the pipeline
    # fills with small groups while the PE p-state is still ramping
    order = np.argsort([len(cs) for _, _, cs in tagged_passes],
                       kind="stable")
    nslots = -(-len(order) // N_CORES)
    # Kg per slot group: max count in the band, rounded up to 16. Leading
    # groups are small (1,1,2) so the first reduces fire ASAP during the
    # DMA/p-state fill; the rest are 4 wide (one PSUM tile).
    def band_max(s0, gs):
        mx = 1
        for sl in range(s0, s0 + gs):
            for r in order[sl * N_CORES:(sl + 1) * N_CORES]:
                mx = max(mx, len(tagged_passes[r][2]))
        return mx
    kgs = []
    s0 = 0
    for gs in (1, 1, 2):
        if s0 >= nslots:
            break
        gs = min(gs, nslots - s0)
        kgs.append((gs, band_max(s0, gs)))
        s0 += gs
    while s0 < nslots:
        gs = min(4, nslots - s0)
        kgs.append((gs, band_max(s0, gs)))
        s0 += gs
    nc = _build_program(tuple(kgs))
    SM = np.zeros((N_CORES, KAUG, nslots, W), BF)
    SM[:, 9:11, :, 0:Q] = BF(1.0)   # stationary ones rows
    SM[:, 9, :, Q:] = BF(np.float32(BIG))  # pad candidates: D' = BIG
    meta = []
    for i, r in enumerate(order):
        pid, qs, cs = tagged_passes[r]
        c, sl = i % N_CORES, i // N_CORES
        a = augs[pid]
        SM[c, :, sl, 0:len(qs)] = a.S[:, qs]
        SM[c, :, sl, Q:Q + len(cs)] = a.M[:, cs]
        meta.append((c, sl, pid, qs))
    in_maps = [{"sm": np.ascontiguousarray(SM[c])} for c in range(N_CORES)]
    br = run_bass_kernel_spmd(nc, in_maps, list(range(N_CORES)))
    LAST_RUN_PROGRAMS.append(nc)
    outs = [np.full(len(a.sq1), np.inf, np.float32) for a in augs]
    for c, sl, pid, qs in meta:
        np.minimum.at(outs[pid], qs, br.results[c]["res"][:len(qs), sl])
    return outs


def kernel(cloud1, cloud2):
    c1 = np.asarray(cloud1, np.float32)
    c2 = np.asarray(cloud2, np.float32)
    LAST_RUN_PROGRAMS.clear()
    # problems: (batch0 1->2, 2->1, batch1 1->2, 2->1)
    probs = []
    for n in range(N):
        probs.append((c1[n], c2[n]))
        probs.append((c2[n], c1[n]))
    augs = [_Aug(A, B) for A, B in probs]

    p1_tagged = []
    g2s = []
    for pid, (A, B) in enumerate(probs):
        passes, g2 = _pack_phase1(A, B)
        g2s.append(g2)
        p1_tagged += [(pid, qs, cs) for qs, cs in passes]
    d1p = _dispatch(p1_tagged, augs)

    p2_tagged = []
    for pid, (A, B) in enumerate(probs):
        d1 = d1p[pid] + augs[pid].sq1
        margin = np.maximum(1e-3 * np.abs(d1), 1e-6)
        fail_idx = np.where(~(d1 + margin <= g2s[pid]))[0]
        rad2 = (d1[fail_idx] + margin[fail_idx]).astype(np.float64)
        p2_tagged += [(pid, qs, cs)
                      for qs, cs in _pack_phase2(A, B, fail_idx, rad2)]
    if p2_tagged:
        d2p = _dispatch(p2_tagged, augs)
    else:
        d2p = [np.full(len(a.sq1), np.inf, np.float32) for a in augs]

    terms = []
    for pid in range(len(probs)):
        dmin = np.minimum(d1p[pid], d2p[pid]) + augs[pid].sq1
        terms.append(dmin.astype(np.float64).mean())
    out = np.array([terms[2 * n] + terms[2 * n + 1] for n in range(N)],
                   np.float32)
    return out
